# revision 15
# baseline (speedup 1.0000x reference)
"""Sparse (shot-local + shared-global) attention on 8 Trainium2 NeuronCores.

Problem: B=2, S_TOT=4096, HD=1024 with H=16 heads (d=64), num_shots=4
(L=1024 tokens per shot), global pool = first 64 tokens of each shot
(G=256), shared by all shots of the same batch element.

Sharding: the 32 (batch, head) pairs are split 4-per-core across 8 cores
(data + head parallel). Each (b,h,shot) block is independent attention of
shape q[1024,64] against k/v[1024+256,64].

v2 design (vs the v1 128x128/64x128 mixed-mode kernel):
  * Every matmul runs in 64x128 row-tiled PE mode - no tiling-mode
    switches (each switch drains the PE array).
  * QK exploits K=D=64: the 128x128 array is split into two 64x128 row
    tiles (T0 = SBUF partitions 0-63, T8 = 64-127).  Two k-slots are
    packed into the two partition halves of kTp, q is duplicated into
    both halves, and the two S^T tile matmuls execute CONCURRENTLY in
    the array (different PSUM banks) - QK costs 256 PE cycles/slot
    instead of 512.
  * PV splits each slot's 128 tokens top/bottom across T0/T8 into two
    accumulators po_A/po_B (merged by DVE in the epilogue).  Same PE
    cycles as unmodeled PV, but stays in 64x128 mode.  v is padded to
    128 weight columns (ones column at 64 emits the softmax denominator
    Z; 63 zero columns keep NumWeights=128 so fast-weight-load applies).
  * S^T PSUM ring of 6 banks; ACT consumes 3-bank [128,1536] groups
    (amortizes the ~352-cycle ACTIVATE overhead); PV lags ACT by 4
    groups (deep SBUF expT backlog keeps the PE busy through HAM
    warm/cold clock oscillation).
  * Softmax max-subtraction skipped: logits ~ N(0,1), exp is in range.

Per-core engine floors (@warm 2.4GHz PE / 1.2GHz ACT): PE 245,760 cyc
= 102us, ACT (163,840 el + 107*352)/1.2GHz = 168us -> ACT-bound.
"""

import sys

sys.path.insert(0, "/opt/trn_rl_repo")

import ml_dtypes
import numpy as np

import concourse.bass as bass  # noqa: F401  (registers AP machinery)
import concourse.mybir as mybir
import concourse.tile as tile
from concourse import bacc
from concourse.bass_utils import run_bass_kernel_spmd

B, S_TOT, HD = 2, 4096, 1024
H, NSHOT, PER_G = 16, 4, 64
D = HD // H            # 64 head dim
L = S_TOT // NSHOT     # 1024 shot length
G = NSHOT * PER_G      # 256 global pool tokens
NCORES = 8
PAIRS = (B * H) // NCORES   # 4 (b,h) pairs per core
QC = 512                    # q chunk width (PSUM bank)
NQC = L // QC               # 2
NSLOT = 10                  # k slots per unit: 8 local + 2 global
NUNIT = PAIRS * NSHOT * NQC  # 32 units/core
NSLOTS_TOT = NUNIT * NSLOT   # 320
RING = 6                    # S^T psum ring banks
GRP = 3                     # slots per ACT group
NGRP = (NSLOTS_TOT + GRP - 1) // GRP  # 107 (last group has 2 slots)
LAG = 4                     # PV lags ACT by this many groups
EXP_BUFS = 10
SCALE = 1.0 / float(np.sqrt(D))
VSLOTS = NSHOT * (L // 128) + G // 128  # 34 v slots per pair

MM_DT = "float16"

_NC = None


def build_program():
    """Build + compile the per-core Bass program (identical on all cores)."""
    global _NC
    if _NC is not None:
        return _NC
    f32 = mybir.dt.float32
    mdt = getattr(mybir.dt, MM_DT)
    Exp = mybir.ActivationFunctionType.Exp

    nc = bacc.Bacc("TRN2", target_bir_lowering=False, debug=True)
    qT_d = nc.dram_tensor("qT", [D, PAIRS, S_TOT], mdt, kind="ExternalInput")
    kTp_d = nc.dram_tensor("kTp", [128, PAIRS, S_TOT // 2], mdt,
                           kind="ExternalInput")
    kgp_d = nc.dram_tensor("kgp", [128, PAIRS, G // 2], mdt,
                           kind="ExternalInput")
    vp_d = nc.dram_tensor("vp", [128, PAIRS, VSLOTS, 65], mdt,
                          kind="ExternalInput")
    oT_d = nc.dram_tensor("oT", [D, PAIRS, S_TOT], f32, kind="ExternalOutput")

    with tile.TileContext(nc) as tc:
        with (
            tc.tile_pool(name="inp", bufs=2) as inp_pool,
            tc.tile_pool(name="work", bufs=2) as work_pool,
            tc.tile_pool(name="ps", bufs=1, space="PSUM") as ps_pool,
        ):
            # Two alternating 3-bank S^T rings (separate tensors: Tile's WAR
            # tracking is tensor-granular, so one 6-bank ring serializes
            # every QK write on the PREVIOUS activate's completion - split
            # tensors restore the intended 2-group runway).
            ringA = ps_pool.tile([128, GRP * QC], f32, tag="ringA", name="ringA")
            ringB = ps_pool.tile([128, GRP * QC], f32, tag="ringB", name="ringB")
            rings = [ringA, ringB]
            po = ps_pool.tile([128, 2 * QC], f32, tag="po", name="po")

            def load_pair(p, first):
                """DMA pair p's inputs; critical slices on the sync queue,
                bulk on the gpsimd queue (parallel issue)."""
                qTd = inp_pool.tile([128, S_TOT], mdt, tag="qTd")
                kTp = inp_pool.tile([128, S_TOT // 2], mdt, tag="kTp")
                kgp = inp_pool.tile([128, G // 2], mdt, tag="kgp")
                vp = inp_pool.tile([128, VSLOTS, 128], mdt, tag="vp")
                nc.sync.dma_start(kTp[:, :QC], kTp_d[:, p, :QC])
                nc.gpsimd.dma_start(qTd[0:64, :QC], qT_d[:, p, :QC])
                nc.gpsimd.dma_start(qTd[64:128, :QC], qT_d[:, p, :QC])
                nc.gpsimd.dma_start(kgp[:], kgp_d[:, p, :])
                nc.gpsimd.dma_start(vp[:, 0:8, 0:65], vp_d[:, p, 0:8, :])
                nc.gpsimd.dma_start(vp[:, 32:34, 0:65], vp_d[:, p, 32:34, :])
                nc.gpsimd.dma_start(qTd[0:64, QC:], qT_d[:, p, QC:])
                nc.gpsimd.dma_start(qTd[64:128, QC:], qT_d[:, p, QC:])
                nc.gpsimd.dma_start(kTp[:, QC:], kTp_d[:, p, QC:])
                nc.gpsimd.dma_start(vp[:, 8:32, 0:65], vp_d[:, p, 8:32, :])
                if first:
                    # one-time zero of the FWL pad columns (the pool slot is
                    # reused by later pairs; pad region is never re-written)
                    nc.vector.memset(vp[:, :, 65:128], 0.0)
                return {"qTd": qTd, "kTp": kTp, "kgp": kgp, "vp": vp}

            sbs = [None] * PAIRS
            sbs[0] = load_pair(0, True)
            sbs[1] = load_pair(1, True)

            def unit_of(s):
                u = s // NSLOT
                return u, u // (NSHOT * NQC), (u % (NSHOT * NQC)) // NQC, u % NQC

            def emit_qk_slot(s):
                """One S^T slot: even slots on T0 (partitions 0-63), odd on
                T8 (64-127).  Adjacent T0/T8 matmuls pair up concurrently in
                the array; emitting per-slot (not per-pair) keeps each
                matmul's ring WAR limited to ITS bank (freed by ACT(g-2)) so
                no QK instruction ever serializes on ACT(g-1)."""
                u, p, shot, qc = unit_of(s)
                sb = sbs[p]
                j = s % NSLOT
                ri, half = j // 2, j % 2
                qcol = shot * L + qc * QC
                lo, hi = (0, 64) if half == 0 else (64, 128)
                if ri < 4:
                    k_lhs = sb["kTp"][lo:hi, shot * QC + ri * 128:
                                      shot * QC + (ri + 1) * 128]
                else:
                    k_lhs = sb["kgp"][lo:hi, :]
                ring = rings[(s // GRP) % 2]
                b0 = (s % GRP) * QC
                nc.tensor.matmul(ring[:, b0:b0 + QC], k_lhs,
                                 sb["qTd"][lo:hi, qcol:qcol + QC],
                                 start=True, stop=True)

            exp_ref = [None] * NSLOTS_TOT

            def emit_act_group(g):
                s0 = GRP * g
                n = min(GRP, NSLOTS_TOT - s0)
                ring = rings[g % 2]
                expT = work_pool.tile([128, GRP * QC], mdt, tag="expT",
                                      bufs=EXP_BUFS)
                nc.scalar.activation(expT[:, 0:n * QC],
                                     ring[:, 0:n * QC],
                                     Exp, scale=SCALE)
                for i in range(n):
                    exp_ref[s0 + i] = (expT, i * QC)

            def emit_pv_slot(s):
                u, p, shot, qc = unit_of(s)
                j = s % NSLOT
                sb = sbs[p]
                vsl = shot * 8 + j if j < 8 else 32 + (j - 8)
                expT, off = exp_ref[s]
                exp_ref[s] = None
                nc.tensor.matmul(po[:, 0:QC], sb["vp"][0:64, vsl, :],
                                 expT[0:64, off:off + QC],
                                 start=(j == 0), stop=(j == NSLOT - 1))
                nc.tensor.matmul(po[:, QC:2 * QC], sb["vp"][64:128, vsl, :],
                                 expT[64:128, off:off + QC],
                                 start=(j == 0), stop=(j == NSLOT - 1))

            def emit_epi(u):
                _, p, shot, qc = (None,) + unit_of(u * NSLOT)[1:]
                qcol = shot * L + qc * QC
                poBs = work_pool.tile([65, QC], f32, tag="poBs")
                nc.vector.tensor_copy(poBs[:], po[0:65, QC:2 * QC])
                o65 = work_pool.tile([65, QC], f32, tag="o65")
                nc.vector.tensor_add(o65[:], po[0:65, 0:QC], poBs[:])
                zsb = work_pool.tile([1, QC], f32, tag="zsb")
                nc.vector.tensor_copy(zsb[:], o65[64:65, :])
                zr = work_pool.tile([1, QC], f32, tag="zr")
                nc.vector.reciprocal_approx_fast(zr[:], zsb[:])
                zb = work_pool.tile([64, QC], f32, tag="zb")
                nc.gpsimd.partition_broadcast(zb[:], zr[:])
                osb = work_pool.tile([64, QC], f32, tag="osb", bufs=8)
                nc.vector.tensor_mul(osb[:], o65[0:64, :], zb[:])
                nc.sync.dma_start(oT_d[:, p, qcol:qcol + QC], osb[:])

            def emit_pv_due(s):
                u, p, _, _ = unit_of(s)
                # prefetch trigger one unit into pair p: by then pair p-1's
                # last PV matmul has executed, so the load's vp WAR is
                # already satisfied and cannot stall the gpsimd queue (which
                # also runs the EPI partition broadcasts).
                if s % (NSLOT * NSHOT * NQC) == NSLOT and 2 <= p + 1 < PAIRS:
                    sbs[p + 1] = load_pair(p + 1, False)
                emit_pv_slot(s)
                if s % NSLOT == NSLOT - 1:
                    emit_epi(u)

            # Per group g the PE-queue order is [QK slots of g][PV(g-LAG)].
            # QK MUST precede PV: the tile lowering gates ACT(g) on a single
            # counting semaphore over all PE matmuls, so any PV matmul
            # emitted before QK(g)'s last slot would serialize into ACT(g)'s
            # wait threshold (measured ~650ns/group of ACT idle).
            pv_next = 0
            for g in range(NGRP):
                for s in range(GRP * g, min(GRP * (g + 1), NSLOTS_TOT)):
                    emit_qk_slot(s)
                emit_act_group(g)
                if g >= LAG:
                    for s in range(pv_next, GRP * (g - LAG + 1)):
                        emit_pv_due(s)
                    pv_next = GRP * (g - LAG + 1)
            for s in range(pv_next, NSLOTS_TOT):
                emit_pv_due(s)
    nc.compile()
    _NC = nc
    return nc


def pack_inputs(q, k, v):
    """Shard + relayout full inputs into per-core input maps."""
    ndt = ml_dtypes.bfloat16 if MM_DT == "bfloat16" else np.float16
    q5 = np.ascontiguousarray(q).reshape(B, S_TOT, H, D)
    k5 = np.ascontiguousarray(k).reshape(B, S_TOT, H, D)
    v5 = np.ascontiguousarray(v).reshape(B, S_TOT, H, D)
    gidx = (np.arange(NSHOT)[:, None] * L + np.arange(PER_G)[None, :]).reshape(-1)

    in_maps = []
    for c in range(NCORES):
        qT = np.empty((D, PAIRS, S_TOT), ndt)
        kTp = np.empty((128, PAIRS, S_TOT // 2), ndt)
        kgp = np.empty((128, PAIRS, G // 2), ndt)
        vp = np.ones((128, PAIRS, VSLOTS, 65), ndt)
        for p in range(PAIRS):
            pair = c * PAIRS + p
            b, h = divmod(pair, H)
            qT[:, p, :] = q5[b, :, h, :].T
            # k slots: [32, 128, 64]; even slots -> partitions 0-63
            ks = k5[b, :, h, :].reshape(-1, 128, D)
            kTp[0:64, p, :] = ks[0::2].transpose(2, 0, 1).reshape(D, -1)
            kTp[64:128, p, :] = ks[1::2].transpose(2, 0, 1).reshape(D, -1)
            kg = k5[b, gidx, h, :].reshape(2, 128, D)
            kgp[0:64, p, :] = kg[0].T
            kgp[64:128, p, :] = kg[1].T
            # v slots: tokens 0-63 -> partitions 0-63, 64-127 -> 64-127
            vs = v5[b, :, h, :].reshape(-1, 128, D)
            vg = v5[b, gidx, h, :].reshape(2, 128, D)
            vall = np.concatenate([vs, vg], 0)  # [34, 128, 64]
            vp[0:64, p, :, 0:64] = vall[:, 0:64].transpose(1, 0, 2)
            vp[64:128, p, :, 0:64] = vall[:, 64:128].transpose(1, 0, 2)
        in_maps.append({"qT": qT, "kTp": kTp, "kgp": kgp, "vp": vp})
    return in_maps


def unpack_outputs(results):
    """Per-core oT [D, PAIRS, S_TOT] -> full [B, S_TOT, HD]."""
    out5 = np.empty((B, S_TOT, H, D), np.float32)
    for c in range(NCORES):
        oT = results[c]["oT"]
        for p in range(PAIRS):
            b, h = divmod(c * PAIRS + p, H)
            out5[b, :, h, :] = oT[:, p, :].T
    return out5.reshape(B, S_TOT, HD)


def kernel(q, k, v, num_heads, num_shots, per_g):
    assert int(num_heads) == H and int(num_shots) == NSHOT and int(per_g) == PER_G
    nc = build_program()
    in_maps = pack_inputs(np.asarray(q), np.asarray(k), np.asarray(v))
    res = run_bass_kernel_spmd(nc, in_maps, list(range(NCORES)))
    return unpack_outputs(res.results)


# revision 19
# speedup vs baseline: 1.2075x; 1.2075x over previous
"""Sparse (shot-local + shared-global) attention on 8 Trainium2 NeuronCores.

Problem: B=2, S_TOT=4096, HD=1024 with H=16 heads (d=64), num_shots=4
(L=1024 tokens per shot), global pool = first 64 tokens of each shot
(G=256), shared by all shots of the same batch element.

Sharding: the 32 (batch, head) pairs are split 4-per-core across 8 cores
(data + head parallel). Each (b,h,shot) block is independent attention of
shape q[1024,64] against k/v[1024+256,64].

v2 design (vs the v1 128x128/64x128 mixed-mode kernel):
  * Every matmul runs in 64x128 row-tiled PE mode - no tiling-mode
    switches (each switch drains the PE array).
  * QK exploits K=D=64: the 128x128 array is split into two 64x128 row
    tiles (T0 = SBUF partitions 0-63, T8 = 64-127).  Two k-slots are
    packed into the two partition halves of kTp, q is duplicated into
    both halves, and the two S^T tile matmuls execute CONCURRENTLY in
    the array (different PSUM banks) - QK costs 256 PE cycles/slot
    instead of 512.
  * PV splits each slot's 128 tokens top/bottom across T0/T8 into two
    accumulators po_A/po_B (merged by DVE in the epilogue).  Same PE
    cycles as unmodeled PV, but stays in 64x128 mode.  v is padded to
    128 weight columns (ones column at 64 emits the softmax denominator
    Z; 63 zero columns keep NumWeights=128 so fast-weight-load applies).
  * S^T PSUM ring of 6 banks; ACT consumes 3-bank [128,1536] groups
    (amortizes the ~352-cycle ACTIVATE overhead); PV lags ACT by 4
    groups (deep SBUF expT backlog keeps the PE busy through HAM
    warm/cold clock oscillation).
  * Softmax max-subtraction skipped: logits ~ N(0,1), exp is in range.

Per-core engine floors (@warm 2.4GHz PE / 1.2GHz ACT): PE 245,760 cyc
= 102us, ACT (163,840 el + 107*352)/1.2GHz = 168us -> ACT-bound.
"""

import sys

sys.path.insert(0, "/opt/trn_rl_repo")

import ml_dtypes
import numpy as np

import concourse.bass as bass  # noqa: F401  (registers AP machinery)
import concourse.mybir as mybir
import concourse.tile as tile
from concourse import bacc
from concourse.bass_utils import run_bass_kernel_spmd

B, S_TOT, HD = 2, 4096, 1024
H, NSHOT, PER_G = 16, 4, 64
D = HD // H            # 64 head dim
L = S_TOT // NSHOT     # 1024 shot length
G = NSHOT * PER_G      # 256 global pool tokens
NCORES = 8
PAIRS = (B * H) // NCORES   # 4 (b,h) pairs per core
QC = 512                    # q chunk width (PSUM bank)
NQC = L // QC               # 2
NSLOT = 10                  # k slots per unit: 8 local + 2 global
NUNIT = PAIRS * NSHOT * NQC  # 32 units/core
NSLOTS_TOT = NUNIT * NSLOT   # 320
RING = 6                    # S^T psum ring banks
GRP = 3                     # slots per ACT group
NGRP = (NSLOTS_TOT + GRP - 1) // GRP  # 107 (last group has 2 slots)
LAG = 4                     # PV lags ACT by this many groups
EXP_BUFS = 12
SCALE = 1.0 / float(np.sqrt(D))
VSLOTS = NSHOT * (L // 128) + G // 128  # 34 v slots per pair

MM_DT = "float16"

_NC = None


def build_program():
    """Build + compile the per-core Bass program (identical on all cores)."""
    global _NC
    if _NC is not None:
        return _NC
    f32 = mybir.dt.float32
    mdt = getattr(mybir.dt, MM_DT)
    Exp = mybir.ActivationFunctionType.Exp

    nc = bacc.Bacc("TRN2", target_bir_lowering=False, debug=True)
    qT_d = nc.dram_tensor("qT", [D, PAIRS, S_TOT], mdt, kind="ExternalInput")
    kTp_d = nc.dram_tensor("kTp", [128, PAIRS, S_TOT // 2], mdt,
                           kind="ExternalInput")
    kgp_d = nc.dram_tensor("kgp", [128, PAIRS, G // 2], mdt,
                           kind="ExternalInput")
    vp_d = nc.dram_tensor("vp", [128, PAIRS, VSLOTS, 65], mdt,
                          kind="ExternalInput")
    oT_d = nc.dram_tensor("oT", [D, PAIRS, S_TOT], f32, kind="ExternalOutput")

    with tile.TileContext(nc) as tc:
        with (
            tc.tile_pool(name="inp", bufs=2) as inp_pool,
            tc.tile_pool(name="work", bufs=2) as work_pool,
            tc.tile_pool(name="ps", bufs=1, space="PSUM") as ps_pool,
        ):
            # Two alternating 3-bank S^T rings (separate tensors: Tile's WAR
            # tracking is tensor-granular, so one 6-bank ring serializes
            # every QK write on the PREVIOUS activate's completion - split
            # tensors restore the intended 2-group runway).
            ringA = ps_pool.tile([128, GRP * QC], f32, tag="ringA", name="ringA")
            ringB = ps_pool.tile([128, GRP * QC], f32, tag="ringB", name="ringB")
            rings = [ringA, ringB]
            po = ps_pool.tile([128, 2 * QC], f32, tag="po", name="po")

            def load_pair(p, first, bulk=None):
                """DMA pair p's inputs.  kTp head on the sync queue; the
                rest on `bulk` (default gpsimd - a queue parallel to sync).
                qTd head covers shot 0's both q-chunks so the first two
                units never wait on the bulk tail."""
                be = bulk if bulk is not None else nc.gpsimd
                qTd = inp_pool.tile([128, S_TOT], mdt, tag="qTd")
                kTp = inp_pool.tile([128, S_TOT // 2], mdt, tag="kTp")
                kgp = inp_pool.tile([128, G // 2], mdt, tag="kgp")
                vp = inp_pool.tile([128, VSLOTS, 128], mdt, tag="vp")
                nc.sync.dma_start(kTp[:, :QC], kTp_d[:, p, :QC])
                be.dma_start(qTd[0:64, :L], qT_d[:, p, :L])
                be.dma_start(qTd[64:128, :L], qT_d[:, p, :L])
                be.dma_start(kgp[:], kgp_d[:, p, :])
                be.dma_start(vp[:, 0:8, 0:65], vp_d[:, p, 0:8, :])
                be.dma_start(vp[:, 32:34, 0:65], vp_d[:, p, 32:34, :])
                be.dma_start(qTd[0:64, L:], qT_d[:, p, L:])
                be.dma_start(qTd[64:128, L:], qT_d[:, p, L:])
                be.dma_start(kTp[:, QC:], kTp_d[:, p, QC:])
                be.dma_start(vp[:, 8:32, 0:65], vp_d[:, p, 8:32, :])
                if first:
                    # one-time zero of the FWL pad columns (the pool slot is
                    # reused by later pairs; pad region is never re-written)
                    nc.vector.memset(vp[:, :, 65:128], 0.0)
                return {"qTd": qTd, "kTp": kTp, "kgp": kgp, "vp": vp}

            sbs = [None] * PAIRS
            sbs[0] = load_pair(0, True)
            # pair 1's bulk goes on the sync queue: the gpsimd queue must
            # stay short so pair 0's tails and the first EPI broadcasts
            # aren't delayed behind 9 more DMA issues.
            sbs[1] = load_pair(1, True, bulk=nc.sync)

            def unit_of(s):
                u = s // NSLOT
                return u, u // (NSHOT * NQC), (u % (NSHOT * NQC)) // NQC, u % NQC

            def emit_qk_slot(s):
                """One S^T slot: even slots on T0 (partitions 0-63), odd on
                T8 (64-127).  Adjacent T0/T8 matmuls pair up concurrently in
                the array; emitting per-slot (not per-pair) keeps each
                matmul's ring WAR limited to ITS bank (freed by ACT(g-2)) so
                no QK instruction ever serializes on ACT(g-1)."""
                u, p, shot, qc = unit_of(s)
                sb = sbs[p]
                j = s % NSLOT
                ri, half = j // 2, j % 2
                qcol = shot * L + qc * QC
                lo, hi = (0, 64) if half == 0 else (64, 128)
                if ri < 4:
                    k_lhs = sb["kTp"][lo:hi, shot * QC + ri * 128:
                                      shot * QC + (ri + 1) * 128]
                else:
                    k_lhs = sb["kgp"][lo:hi, :]
                ring = rings[(s // GRP) % 2]
                b0 = (s % GRP) * QC
                nc.tensor.matmul(ring[:, b0:b0 + QC], k_lhs,
                                 sb["qTd"][lo:hi, qcol:qcol + QC],
                                 start=True, stop=True)

            exp_ref = [None] * NSLOTS_TOT

            def emit_act_group(g):
                s0 = GRP * g
                n = min(GRP, NSLOTS_TOT - s0)
                ring = rings[g % 2]
                expT = work_pool.tile([128, GRP * QC], mdt, tag="expT",
                                      bufs=EXP_BUFS)
                nc.scalar.activation(expT[:, 0:n * QC],
                                     ring[:, 0:n * QC],
                                     Exp, scale=SCALE)
                for i in range(n):
                    exp_ref[s0 + i] = (expT, i * QC)

            def emit_pv_slot(s):
                u, p, shot, qc = unit_of(s)
                j = s % NSLOT
                sb = sbs[p]
                vsl = shot * 8 + j if j < 8 else 32 + (j - 8)
                expT, off = exp_ref[s]
                exp_ref[s] = None
                nc.tensor.matmul(po[:, 0:QC], sb["vp"][0:64, vsl, :],
                                 expT[0:64, off:off + QC],
                                 start=(j == 0), stop=(j == NSLOT - 1))
                nc.tensor.matmul(po[:, QC:2 * QC], sb["vp"][64:128, vsl, :],
                                 expT[64:128, off:off + QC],
                                 start=(j == 0), stop=(j == NSLOT - 1))

            def emit_epi(u):
                _, p, shot, qc = (None,) + unit_of(u * NSLOT)[1:]
                qcol = shot * L + qc * QC
                poBs = work_pool.tile([65, QC], f32, tag="poBs")
                nc.vector.tensor_copy(poBs[:], po[0:65, QC:2 * QC])
                o65 = work_pool.tile([65, QC], f32, tag="o65")
                nc.vector.tensor_add(o65[:], po[0:65, 0:QC], poBs[:])
                zsb = work_pool.tile([1, QC], f32, tag="zsb")
                nc.vector.tensor_copy(zsb[:], o65[64:65, :])
                zr = work_pool.tile([1, QC], f32, tag="zr")
                nc.vector.reciprocal_approx_fast(zr[:], zsb[:])
                zb = work_pool.tile([64, QC], f32, tag="zb")
                nc.gpsimd.partition_broadcast(zb[:], zr[:])
                osb = work_pool.tile([64, QC], f32, tag="osb", bufs=8)
                nc.vector.tensor_mul(osb[:], o65[0:64, :], zb[:])
                nc.sync.dma_start(oT_d[:, p, qcol:qcol + QC], osb[:])

            def emit_pv_due(s):
                u, p, _, _ = unit_of(s)
                # prefetch trigger one unit into pair p: by then pair p-1's
                # last PV matmul has executed, so the load's vp WAR is
                # already satisfied and cannot stall the gpsimd queue (which
                # also runs the EPI partition broadcasts).
                if s % (NSLOT * NSHOT * NQC) == NSLOT and 2 <= p + 1 < PAIRS:
                    sbs[p + 1] = load_pair(p + 1, False)
                emit_pv_slot(s)
                if s % NSLOT == NSLOT - 1:
                    emit_epi(u)

            # Per group g the PE-queue order is [QK slots of g][PV(g-LAG)].
            # QK MUST precede PV: the tile lowering gates ACT(g) on a single
            # counting semaphore over all PE matmuls, so any PV matmul
            # emitted before QK(g)'s last slot would serialize into ACT(g)'s
            # wait threshold (measured ~650ns/group of ACT idle).
            pv_next = 0
            for g in range(NGRP):
                for s in range(GRP * g, min(GRP * (g + 1), NSLOTS_TOT)):
                    emit_qk_slot(s)
                emit_act_group(g)
                # shrink the PV lag over the last groups so the post-ACT
                # drain tail is ~1 group of PV instead of LAG groups
                lag = LAG if g < NGRP - 4 else 1
                if g >= lag:
                    for s in range(pv_next, GRP * (g - lag + 1)):
                        emit_pv_due(s)
                    pv_next = GRP * (g - lag + 1)
            for s in range(pv_next, NSLOTS_TOT):
                emit_pv_due(s)
    nc.compile()
    _NC = nc
    return nc


def pack_inputs(q, k, v):
    """Shard + relayout full inputs into per-core input maps."""
    ndt = ml_dtypes.bfloat16 if MM_DT == "bfloat16" else np.float16
    q5 = np.ascontiguousarray(q).reshape(B, S_TOT, H, D)
    k5 = np.ascontiguousarray(k).reshape(B, S_TOT, H, D)
    v5 = np.ascontiguousarray(v).reshape(B, S_TOT, H, D)
    gidx = (np.arange(NSHOT)[:, None] * L + np.arange(PER_G)[None, :]).reshape(-1)

    in_maps = []
    for c in range(NCORES):
        qT = np.empty((D, PAIRS, S_TOT), ndt)
        kTp = np.empty((128, PAIRS, S_TOT // 2), ndt)
        kgp = np.empty((128, PAIRS, G // 2), ndt)
        vp = np.ones((128, PAIRS, VSLOTS, 65), ndt)
        for p in range(PAIRS):
            pair = c * PAIRS + p
            b, h = divmod(pair, H)
            qT[:, p, :] = q5[b, :, h, :].T
            # k slots: [32, 128, 64]; even slots -> partitions 0-63
            ks = k5[b, :, h, :].reshape(-1, 128, D)
            kTp[0:64, p, :] = ks[0::2].transpose(2, 0, 1).reshape(D, -1)
            kTp[64:128, p, :] = ks[1::2].transpose(2, 0, 1).reshape(D, -1)
            kg = k5[b, gidx, h, :].reshape(2, 128, D)
            kgp[0:64, p, :] = kg[0].T
            kgp[64:128, p, :] = kg[1].T
            # v slots: tokens 0-63 -> partitions 0-63, 64-127 -> 64-127
            vs = v5[b, :, h, :].reshape(-1, 128, D)
            vg = v5[b, gidx, h, :].reshape(2, 128, D)
            vall = np.concatenate([vs, vg], 0)  # [34, 128, 64]
            vp[0:64, p, :, 0:64] = vall[:, 0:64].transpose(1, 0, 2)
            vp[64:128, p, :, 0:64] = vall[:, 64:128].transpose(1, 0, 2)
        in_maps.append({"qT": qT, "kTp": kTp, "kgp": kgp, "vp": vp})
    return in_maps


def unpack_outputs(results):
    """Per-core oT [D, PAIRS, S_TOT] -> full [B, S_TOT, HD]."""
    out5 = np.empty((B, S_TOT, H, D), np.float32)
    for c in range(NCORES):
        oT = results[c]["oT"]
        for p in range(PAIRS):
            b, h = divmod(c * PAIRS + p, H)
            out5[b, :, h, :] = oT[:, p, :].T
    return out5.reshape(B, S_TOT, HD)


def kernel(q, k, v, num_heads, num_shots, per_g):
    assert int(num_heads) == H and int(num_shots) == NSHOT and int(per_g) == PER_G
    nc = build_program()
    in_maps = pack_inputs(np.asarray(q), np.asarray(k), np.asarray(v))
    res = run_bass_kernel_spmd(nc, in_maps, list(range(NCORES)))
    return unpack_outputs(res.results)
